# revision 18
# baseline (speedup 1.0000x reference)
"""Multi-head attention (B=4, S=2048, D=1024, H=16) on 8 TRN2 NeuronCores.

Sharding: pure tensor-parallel over heads. Core c owns heads (2c, 2c+1) of
EVERY batch — i.e. columns [128c, 128(c+1)) of Wq/Wk/Wv and the matching
128 rows of Wo. Attention work per (batch, head) scales with
nk_b = ceil(valid_len_b / 128) key tiles, so giving each core 2 heads of
every batch balances the per-core load exactly (each core does
2 * sum_b nk_b head-keytile units) no matter how skewed valid_lens are.
Row-parallel Wo produces per-core partial outputs [B*S, D] (bf16); the
host sums the 8 partials.

Per-core dataflow, per batch b (kt counts specialized at build time):
  QT_b[d',q] = (Xq_b Wq_c)^T   bf16, head dims on partitions (h0: 0:64,
                               h1: 64:128)
  KT_b[d',k] = (Xk_b Wk_c)^T   only the first nk_b*128 key positions
  V_b[k, h, 65]                (Xv_b Wv_c) + per-head ones column (softmax
                               denominator rides along row 64 of av)
  per 512-wide q chunk:
    per kt: scoresT[k,q] for both heads (PE row groups 0/64 -> concurrent)
            ex = exp(scores * scale + maskbias)   one ACT instr, both heads
            av_h[65, q] += V_h^T-ish @ ex_h       (PSUM accumulate over kt)
    rc = 1/av[64]  (denominators, DVE reciprocal straight from PSUM)
    bc = ones^T @ rc             broadcast 1/denom to 128 partitions (PE)
    OT[0:64] = av_h0 * bc_h0 ; t1 = av_h1 * bc_h1  (DVE mul, PSUM inputs)
    OT[64:128] <- t1  via SBUF->SBUF DMA (partition move: engines cannot
                     shift partition base, DMA can)
  out_b = OT^T-slices @ Wo_c   full 128-dim contraction (both heads in one
                               accumulation-free matmul pair per q tile)

Masking is pure data: mb/ms [128, sum nk_b] hold per-key exp bias/scale;
valid_len==0 gives scale=0,bias=0 -> uniform attention over all S keys,
matching jax.nn.softmax of an all-masked row.
"""

import math

import numpy as np

B, S, D, H = 4, 2048, 1024, 16
HD = D // H  # 64
NCORES = 8
NEG = -1.0e6
P = 128

_PROG_CACHE = {}


def _patch_tile_drain():
    """The walrus build in this container rejects sem waits attached to the
    Tile end-of-kernel Drain ("Too many sync wait commands" / SIGABRT).
    Replace them with standalone EventSemaphore waits, which it accepts."""
    import concourse.tile as tile
    from concourse.vector_clock import ScopedClock

    if getattr(tile.TileContext, "_drain_patched", False):
        return

    def _drain_and_barrier(self, tick_clock, wait_clock):
        nc = self.nc
        drain_inst = nc.sync.drain()
        wait_clock.add_sem_waits(
            drain_inst.ins, ScopedClock({None: tick_clock.global_clock})
        )
        si = drain_inst.ins.sync_info
        waits = list(si.on_wait) if si is not None and si.on_wait else []
        if waits:
            si.on_wait.clear()
            by_id, by_name = {}, {}
            for h in wait_clock.sems.allocated().values():
                by_id[getattr(h, "id", None)] = h
                by_name[getattr(h, "name", None)] = h
            for w in waits:
                h = by_id.get(w.id) or by_name.get(w.ant_name)
                assert h is not None, f"no handle for sem {w.ant_name} ({w.id})"
                nc.sync.wait_ge(h, w.wait_value)
        nc.all_engine_barrier()
        assert self.sems is not None
        popped = nc._tile_sem_poison_stack.pop()
        assert popped is self._sem_poison
        nc.clear_and_free_semaphores(list(self.sems.allocated().values()))
        nc.all_engine_barrier()

    tile.TileContext._drain_and_barrier = _drain_and_barrier
    tile.TileContext._drain_patched = True


def _split_multi_waits(nc, mybir):
    """This container's walrus rejects instructions carrying more than one
    semaphore wait ("Too many sync wait commands"). Hoist excess waits into
    standalone EventSemaphore instructions on the same engine, inserted
    immediately before the instruction — same-engine stream order preserves
    the semantics exactly."""
    n_ev = 0
    for fn in nc.m.functions:
        for bb in fn.blocks:
            insts = bb.instructions
            out = []
            for inst in insts:
                si = inst.sync_info
                waits = list(si.on_wait) if si is not None and si.on_wait else []
                keep = 0 if inst.opcode == "Drain" else 1
                if len(waits) > keep:
                    excess = waits[: len(waits) - keep]
                    kept = waits[len(waits) - keep:]
                    si.on_wait.clear()
                    si.on_wait.extend(kept)
                    for w in excess:
                        ev = mybir.InstEventSemaphore(
                            name=f"{inst.name}-hw{n_ev}",
                            engine=inst.engine,
                        )
                        ev.sync_info = mybir.SyncInfo(on_wait=[w], on_update=[])
                        out.append(ev)
                        n_ev += 1
                out.append(inst)
            if n_ev:
                insts[:] = out
    return n_ev


def _build_program(nks: tuple):
    import concourse.bass as bass
    import concourse.mybir as mybir
    import concourse.tile as tile

    _patch_tile_drain()

    f32 = mybir.dt.float32
    f32r = mybir.dt.float32r
    bf16 = mybir.dt.bfloat16
    AF = mybir.ActivationFunctionType

    KT_tot = sum(nks)
    SK = KT_tot * P
    offs = [sum(nks[:b]) for b in range(B)]

    nc = bass.Bass()

    xq_d = nc.dram_tensor("xq", [B, D, S], bf16, kind="ExternalInput")
    xk_d = nc.dram_tensor("xk", [D, SK], bf16, kind="ExternalInput")
    xv_d = nc.dram_tensor("xv", [D, SK], bf16, kind="ExternalInput")
    wq_d = nc.dram_tensor("wq", [D, P], bf16, kind="ExternalInput")
    wk_d = nc.dram_tensor("wk", [D, P], bf16, kind="ExternalInput")
    wv_d = nc.dram_tensor("wv", [D, P], bf16, kind="ExternalInput")
    wo_d = nc.dram_tensor("wo", [P, D], bf16, kind="ExternalInput")
    mb_d = nc.dram_tensor("mb", [P, KT_tot], f32, kind="ExternalInput")
    ms_d = nc.dram_tensor("ms", [P, KT_tot], f32, kind="ExternalInput")
    out_d = nc.dram_tensor("out", [B * S, D], bf16, kind="ExternalOutput")

    with tile.TileContext(nc) as tc:
        with (
            tc.tile_pool(name="pp", bufs=1) as pp,
            tc.tile_pool(name="qtp", bufs=2) as qtp,
            tc.tile_pool(name="ktp", bufs=2) as ktp,
            tc.tile_pool(name="vp", bufs=2) as vp,
            tc.tile_pool(name="otp", bufs=2) as otp,
            tc.tile_pool(name="t1p", bufs=3) as t1p,
            tc.tile_pool(name="xtp", bufs=3) as xtp,
            tc.tile_pool(name="expp", bufs=3) as expp,
            tc.tile_pool(name="rcpp", bufs=2) as rcpp,
            tc.tile_pool(name="bcsp", bufs=2) as bcsp,
            tc.tile_pool(name="outp", bufs=3) as outp,
            tc.tile_pool(name="psA", bufs=2, space="PSUM") as psA,
            tc.tile_pool(name="psB", bufs=2, space="PSUM") as psB,
        ):
            # persistent: weights, masks, ones row
            wq = pp.tile([P, 8, P], bf16, name="wq")
            wk = pp.tile([P, 8, P], bf16, name="wk")
            wv = pp.tile([P, 8, P], bf16, name="wv")
            wo = pp.tile([P, D], bf16, name="wo")
            mb = pp.tile([P, KT_tot], f32, name="mb")
            msc = pp.tile([P, KT_tot], f32, name="msc")

            nc.sync.dma_start(wq[:], wq_d[:, :].rearrange("(a p) c -> p a c", p=P))
            nc.sync.dma_start(wk[:], wk_d[:, :].rearrange("(a p) c -> p a c", p=P))
            nc.sync.dma_start(wv[:], wv_d[:, :].rearrange("(a p) c -> p a c", p=P))
            nc.sync.dma_start(wo[:], wo_d[:, :])
            nc.sync.dma_start(mb[:], mb_d[:, :])
            nc.sync.dma_start(msc[:], ms_d[:, :])
            ones1 = pp.tile([1, P], bf16, name="ones1")
            nc.any.memset(ones1[:], 1.0)

            border = sorted(range(B), key=lambda bb: -nks[bb])
            for b in border:
                nk = nks[b]
                off = offs[b]
                skb = nk * P

                # ---- Phase A: QT, KT projections (head dims on partitions).
                # a-outer so each weight slice is loaded once per chunk-pair
                # (LDWEIGHTS is ~107ns per 128-col load; a-inner pays it on
                # every accumulation step).
                QT = qtp.tile([P, S], bf16, name=f"QT{b}", tag="qt")
                xq_re = xq_d[b].rearrange("(a p) s -> p a s", p=P)
                KT = ktp.tile([P, skb], bf16, name=f"KT{b}", tag="kt")
                xk_re = xk_d[:, off * P:off * P + skb].rearrange(
                    "(a p) s -> p a s", p=P
                )
                for (w_sb, x_re, dst, tot) in (
                    (wq, xq_re, QT, S),
                    (wk, xk_re, KT, skb),
                ):
                    chunks = [
                        (s0, min(512, tot - s0)) for s0 in range(0, tot, 512)
                    ]
                    for pair0 in range(0, len(chunks), 2):
                        pch = chunks[pair0:pair0 + 2]
                        xss = []
                        for (s0, w) in pch:
                            xs = xtp.tile([P, 8, 512], bf16, name="xs", tag="xt")
                            nc.sync.dma_start(
                                xs[:, :, 0:w], x_re[:, :, s0:s0 + w]
                            )
                            xss.append(xs)
                        pj = psA.tile([P, 2, 512], f32, name="pj", tag="A")
                        for a in range(8):
                            for j, (s0, w) in enumerate(pch):
                                nc.tensor.matmul(
                                    pj[:, j, 0:w],
                                    lhsT=w_sb[:, a, :],
                                    rhs=xss[j][:, a, 0:w],
                                    start=(a == 0),
                                    stop=(a == 7),
                                )
                        for j, (s0, w) in enumerate(pch):
                            nc.vector.tensor_copy(
                                out=dst[:, s0:s0 + w], in_=pj[:, j, 0:w]
                            )

                # ---- Phase B: V projection (keys on partitions, + ones col)
                V = vp.tile([P, nk, 2, HD + 1], bf16, name=f"V{b}", tag="v")
                xv_re = xv_d[:, off * P:off * P + skb].rearrange(
                    "(a p) s -> p a s", p=P
                )
                for kt in range(nk):
                    xvt = xtp.tile([P, 8, P], bf16, name="xvt", tag="xvt")
                    nc.sync.dma_start(xvt[:], xv_re[:, :, kt * P:(kt + 1) * P])
                    pv = psA.tile([P, P], f32, name="pv", tag="A")
                    for a in range(8):
                        nc.tensor.matmul(
                            pv[:],
                            lhsT=xvt[:, a, :],
                            rhs=wv[:, a, :],
                            start=(a == 0),
                            stop=(a == 7),
                        )
                    nc.any.memset(V[:, kt, :, HD:HD + 1], 1.0)
                    nc.scalar.copy(
                        out=V[:, kt, :, 0:HD],
                        in_=pv[:].rearrange("p (h c) -> p h c", c=HD),
                    )

                # ---- Phase C: attention for both heads, 512-wide q chunks,
                # with the output projection of each finished q chunk inlined
                # (fills the ACT-bound kt-loop gaps, spreads the out-DMA).
                OT = otp.tile([P, S], bf16, name=f"OT{b}", tag="ot")
                for qh in range(4):
                    q0 = qh * 512
                    avs = psB.tile([HD + 1, 2, 512], f32, name="avs", tag="av")
                    for kt in range(nk):
                        scs = psA.tile([P, 2, 512], f32, name="scs", tag="A")
                        for h in range(2):
                            pb = h * HD
                            nc.tensor.matmul(
                                scs[:, h, :],
                                lhsT=KT[pb:pb + HD, kt * P:(kt + 1) * P],
                                rhs=QT[pb:pb + HD, q0:q0 + 512],
                                start=True,
                                stop=True,
                            )
                        ex = expp.tile([P, 2, 512], bf16, name="ex", tag="ex")
                        nc.scalar.activation(
                            ex[:],
                            scs[:],
                            AF.Exp,
                            bias=mb[:, off + kt:off + kt + 1],
                            scale=msc[:, off + kt:off + kt + 1],
                        )
                        for h in range(2):
                            nc.tensor.matmul(
                                avs[:, h, :],
                                lhsT=V[:, kt, h, :],
                                rhs=ex[:, h, :],
                                start=(kt == 0),
                                stop=(kt == nk - 1),
                            )
                    # 1/denom as exp(-ln(denom)): InstReciprocal is ~6.5
                    # ns/elem on DVE and the custom-DVE fast recip doesn't
                    # encode under this walrus; two table ACT passes are 5x
                    # cheaper and accurate to ~1e-3.
                    lnd = rcpp.tile([1, 2, 512], f32, name="lnd", tag="rc")
                    nc.scalar.activation(
                        lnd[:].rearrange("p a b -> p (a b)"),
                        avs[HD:HD + 1, :, :].rearrange("p a b -> p (a b)"),
                        AF.Ln,
                    )
                    rcb = rcpp.tile([1, 2, 512], bf16, name="rcb", tag="rcb")
                    nc.scalar.activation(rcb[:], lnd[:], AF.Exp, scale=-1.0)
                    bc = psA.tile([P, 2, 512], f32, name="bc", tag="A")
                    for h in range(2):
                        nc.tensor.matmul(
                            bc[:, h, :],
                            lhsT=ones1[:],
                            rhs=rcb[:, h, :],
                            start=True,
                            stop=True,
                        )
                    bcs = bcsp.tile([P, 2, 512], bf16, name="bcs", tag="bcs")
                    nc.vector.tensor_copy(out=bcs[:], in_=bc[:])
                    nc.vector.tensor_mul(
                        out=OT[0:HD, q0:q0 + 512],
                        in0=avs[0:HD, 0, :],
                        in1=bcs[0:HD, 0, :],
                    )
                    t1 = t1p.tile([HD, 512], bf16, name="t1", tag="t1")
                    nc.vector.tensor_mul(
                        out=t1[:],
                        in0=avs[0:HD, 1, :],
                        in1=bcs[0:HD, 1, :],
                    )
                    # engines cannot write across partition bases; DMA can
                    nc.sync.dma_start(OT[HD:P, q0:q0 + 512], t1[:])

                    # output projection for this q chunk (full 128-dim
                    # contraction: both heads in one matmul per 512 slab)
                    for qt in range(qh * 4, qh * 4 + 4):
                        wps = psA.tile([P, D], f32, name="wps", tag="A")
                        for ch2 in range(2):
                            nc.tensor.matmul(
                                wps[:, ch2 * 512:(ch2 + 1) * 512],
                                lhsT=OT[:, qt * P:(qt + 1) * P],
                                rhs=wo[:, ch2 * 512:(ch2 + 1) * 512],
                                start=True,
                                stop=True,
                            )
                        ob = outp.tile([P, D], bf16, name="ob", tag="ob")
                        if qt % 2 == 0:
                            nc.vector.tensor_copy(out=ob[:], in_=wps[:])
                        else:
                            nc.scalar.copy(out=ob[:], in_=wps[:])
                        nc.sync.dma_start(
                            out_d[(b * 16 + qt) * P:(b * 16 + qt + 1) * P, :],
                            ob[:],
                        )

    _split_multi_waits(nc, mybir)
    return nc


def _get_program(nks: tuple):
    if nks not in _PROG_CACHE:
        _PROG_CACHE[nks] = _build_program(nks)
    return _PROG_CACHE[nks]


def kernel(**inputs) -> np.ndarray:
    import ml_dtypes
    from concourse.bass_utils import run_bass_kernel_spmd

    bf = ml_dtypes.bfloat16

    q = np.asarray(inputs["queries"], dtype=np.float32)
    k = np.asarray(inputs["keys"], dtype=np.float32)
    v = np.asarray(inputs["values"], dtype=np.float32)
    vl = np.asarray(inputs["valid_lens"]).astype(np.int64)
    Wq = np.asarray(inputs["Wq"], dtype=np.float32)
    Wk = np.asarray(inputs["Wk"], dtype=np.float32)
    Wv = np.asarray(inputs["Wv"], dtype=np.float32)
    Wo = np.asarray(inputs["Wo"], dtype=np.float32)

    nks = tuple(
        (S // P) if int(vl[b]) == 0
        else min(S // P, int(math.ceil(int(vl[b]) / P)))
        for b in range(B)
    )
    nc = _get_program(nks)

    KT_tot = sum(nks)
    offs = [sum(nks[:b]) for b in range(B)]

    # shared across cores (host arrays reused; staging per device is free)
    xq = np.ascontiguousarray(q.transpose(0, 2, 1)).astype(bf)
    xk = np.concatenate(
        [k[b].T[:, : nks[b] * P] for b in range(B)], axis=1
    ).astype(bf)
    xv = np.concatenate(
        [v[b].T[:, : nks[b] * P] for b in range(B)], axis=1
    ).astype(bf)

    m_bias = np.empty((P, KT_tot), np.float32)
    m_scale = np.empty((P, KT_tot), np.float32)
    for b in range(B):
        vlb = int(vl[b])
        kk = (
            np.arange(nks[b])[None, :] * P + np.arange(P)[:, None]
        ).astype(np.int64)
        if vlb == 0:
            m_bias[:, offs[b]:offs[b] + nks[b]] = 0.0
            m_scale[:, offs[b]:offs[b] + nks[b]] = 0.0
        else:
            m_bias[:, offs[b]:offs[b] + nks[b]] = np.where(kk < vlb, 0.0, NEG)
            m_scale[:, offs[b]:offs[b] + nks[b]] = 1.0 / math.sqrt(HD)

    in_maps = []
    for c in range(NCORES):
        cols = slice(c * P, (c + 1) * P)
        in_maps.append(
            {
                "xq": xq,
                "xk": xk,
                "xv": xv,
                "wq": np.ascontiguousarray(Wq[:, cols]).astype(bf),
                "wk": np.ascontiguousarray(Wk[:, cols]).astype(bf),
                "wv": np.ascontiguousarray(Wv[:, cols]).astype(bf),
                "wo": np.ascontiguousarray(Wo[cols, :]).astype(bf),
                "mb": m_bias,
                "ms": m_scale,
            }
        )

    globals()["_LAST_IN_MAPS"] = in_maps
    res = run_bass_kernel_spmd(nc, in_maps, list(range(NCORES))).results

    acc = np.zeros((B * S, D), dtype=np.float32)
    for c in range(NCORES):
        acc += res[c]["out"].astype(np.float32)
    return acc.reshape(B, S, D)


# revision 20
# speedup vs baseline: 1.1780x; 1.1780x over previous
"""Multi-head attention (B=4, S=2048, D=1024, H=16) on 8 TRN2 NeuronCores.

Sharding: pure tensor-parallel over heads. Core c owns heads (2c, 2c+1) of
EVERY batch — i.e. columns [128c, 128(c+1)) of Wq/Wk/Wv and the matching
128 rows of Wo. Attention work per (batch, head) scales with
nk_b = ceil(valid_len_b / 128) key tiles, so giving each core 2 heads of
every batch balances the per-core load exactly (each core does
2 * sum_b nk_b head-keytile units) no matter how skewed valid_lens are.
Row-parallel Wo produces per-core partial outputs [B*S, D] (bf16); the
host sums the 8 partials.

Per-core dataflow, per batch b (kt counts specialized at build time):
  QT_b[d',q] = (Xq_b Wq_c)^T   bf16, head dims on partitions (h0: 0:64,
                               h1: 64:128)
  KT_b[d',k] = (Xk_b Wk_c)^T   only the first nk_b*128 key positions
  V_b[k, h, 65]                (Xv_b Wv_c) + per-head ones column (softmax
                               denominator rides along row 64 of av)
  per 512-wide q chunk:
    per kt: scoresT[k,q] for both heads (PE row groups 0/64 -> concurrent)
            ex = exp(scores * scale + maskbias)   one ACT instr, both heads
            av_h[65, q] += V_h^T-ish @ ex_h       (PSUM accumulate over kt)
    rc = 1/av[64]  (denominators, DVE reciprocal straight from PSUM)
    bc = ones^T @ rc             broadcast 1/denom to 128 partitions (PE)
    OT[0:64] = av_h0 * bc_h0 ; t1 = av_h1 * bc_h1  (DVE mul, PSUM inputs)
    OT[64:128] <- t1  via SBUF->SBUF DMA (partition move: engines cannot
                     shift partition base, DMA can)
  out_b = OT^T-slices @ Wo_c   full 128-dim contraction (both heads in one
                               accumulation-free matmul pair per q tile)

Masking is pure data: mb/ms [128, sum nk_b] hold per-key exp bias/scale;
valid_len==0 gives scale=0,bias=0 -> uniform attention over all S keys,
matching jax.nn.softmax of an all-masked row.
"""

import math

import numpy as np

B, S, D, H = 4, 2048, 1024, 16
HD = D // H  # 64
NCORES = 8
NEG = -1.0e6
P = 128

_PROG_CACHE = {}


def _patch_tile_drain():
    """The walrus build in this container rejects sem waits attached to the
    Tile end-of-kernel Drain ("Too many sync wait commands" / SIGABRT).
    Replace them with standalone EventSemaphore waits, which it accepts."""
    import concourse.tile as tile
    from concourse.vector_clock import ScopedClock

    if getattr(tile.TileContext, "_drain_patched", False):
        return

    def _drain_and_barrier(self, tick_clock, wait_clock):
        nc = self.nc
        drain_inst = nc.sync.drain()
        wait_clock.add_sem_waits(
            drain_inst.ins, ScopedClock({None: tick_clock.global_clock})
        )
        si = drain_inst.ins.sync_info
        waits = list(si.on_wait) if si is not None and si.on_wait else []
        if waits:
            si.on_wait.clear()
            by_id, by_name = {}, {}
            for h in wait_clock.sems.allocated().values():
                by_id[getattr(h, "id", None)] = h
                by_name[getattr(h, "name", None)] = h
            for w in waits:
                h = by_id.get(w.id) or by_name.get(w.ant_name)
                assert h is not None, f"no handle for sem {w.ant_name} ({w.id})"
                nc.sync.wait_ge(h, w.wait_value)
        nc.all_engine_barrier()
        assert self.sems is not None
        popped = nc._tile_sem_poison_stack.pop()
        assert popped is self._sem_poison
        nc.clear_and_free_semaphores(list(self.sems.allocated().values()))
        nc.all_engine_barrier()

    tile.TileContext._drain_and_barrier = _drain_and_barrier
    tile.TileContext._drain_patched = True


def _split_multi_waits(nc, mybir):
    """This container's walrus rejects instructions carrying more than one
    semaphore wait ("Too many sync wait commands"). Hoist excess waits into
    standalone EventSemaphore instructions on the same engine, inserted
    immediately before the instruction — same-engine stream order preserves
    the semantics exactly."""
    n_ev = 0
    for fn in nc.m.functions:
        for bb in fn.blocks:
            insts = bb.instructions
            out = []
            for inst in insts:
                si = inst.sync_info
                waits = list(si.on_wait) if si is not None and si.on_wait else []
                keep = 0 if inst.opcode == "Drain" else 1
                if len(waits) > keep:
                    excess = waits[: len(waits) - keep]
                    kept = waits[len(waits) - keep:]
                    si.on_wait.clear()
                    si.on_wait.extend(kept)
                    for w in excess:
                        ev = mybir.InstEventSemaphore(
                            name=f"{inst.name}-hw{n_ev}",
                            engine=inst.engine,
                        )
                        ev.sync_info = mybir.SyncInfo(on_wait=[w], on_update=[])
                        out.append(ev)
                        n_ev += 1
                out.append(inst)
            if n_ev:
                insts[:] = out
    return n_ev


def _build_program(nks: tuple):
    import concourse.bass as bass
    import concourse.mybir as mybir
    import concourse.tile as tile

    _patch_tile_drain()

    f32 = mybir.dt.float32
    f32r = mybir.dt.float32r
    bf16 = mybir.dt.bfloat16
    AF = mybir.ActivationFunctionType

    KT_tot = sum(nks)
    SK = KT_tot * P
    offs = [sum(nks[:b]) for b in range(B)]

    nc = bass.Bass()

    xq_d = nc.dram_tensor("xq", [B, D, S], bf16, kind="ExternalInput")
    xk_d = nc.dram_tensor("xk", [D, SK], bf16, kind="ExternalInput")
    xv_d = nc.dram_tensor("xv", [D, SK], bf16, kind="ExternalInput")
    wq_d = nc.dram_tensor("wq", [D, P], bf16, kind="ExternalInput")
    wk_d = nc.dram_tensor("wk", [D, P], bf16, kind="ExternalInput")
    wv_d = nc.dram_tensor("wv", [D, P], bf16, kind="ExternalInput")
    wo_d = nc.dram_tensor("wo", [P, D], bf16, kind="ExternalInput")
    mb_d = nc.dram_tensor("mb", [P, KT_tot], f32, kind="ExternalInput")
    ms_d = nc.dram_tensor("ms", [P, KT_tot], f32, kind="ExternalInput")
    out_d = nc.dram_tensor("out", [B * S, D], bf16, kind="ExternalOutput")

    with tile.TileContext(nc) as tc:
        with (
            tc.tile_pool(name="pp", bufs=1) as pp,
            tc.tile_pool(name="qtp", bufs=2) as qtp,
            tc.tile_pool(name="ktp", bufs=2) as ktp,
            tc.tile_pool(name="vp", bufs=2) as vp,
            tc.tile_pool(name="otp", bufs=2) as otp,
            tc.tile_pool(name="t1p", bufs=3) as t1p,
            tc.tile_pool(name="xtp", bufs=3) as xtp,
            tc.tile_pool(name="expp", bufs=3) as expp,
            tc.tile_pool(name="rcpp", bufs=2) as rcpp,
            tc.tile_pool(name="bcsp", bufs=2) as bcsp,
            tc.tile_pool(name="outp", bufs=3) as outp,
            tc.tile_pool(name="psA", bufs=2, space="PSUM") as psA,
            tc.tile_pool(name="psB", bufs=2, space="PSUM") as psB,
        ):
            # persistent: weights, masks, ones row
            wq = pp.tile([P, 8, P], bf16, name="wq")
            wk = pp.tile([P, 8, P], bf16, name="wk")
            wv = pp.tile([P, 8, P], bf16, name="wv")
            wo = pp.tile([P, D], bf16, name="wo")
            mb = pp.tile([P, KT_tot], f32, name="mb")
            msc = pp.tile([P, KT_tot], f32, name="msc")

            nc.sync.dma_start(wq[:], wq_d[:, :].rearrange("(a p) c -> p a c", p=P))
            nc.sync.dma_start(wk[:], wk_d[:, :].rearrange("(a p) c -> p a c", p=P))
            nc.sync.dma_start(wv[:], wv_d[:, :].rearrange("(a p) c -> p a c", p=P))
            nc.sync.dma_start(wo[:], wo_d[:, :])
            nc.sync.dma_start(mb[:], mb_d[:, :])
            nc.sync.dma_start(msc[:], ms_d[:, :])
            ones1 = pp.tile([1, P], bf16, name="ones1")
            nc.any.memset(ones1[:], 1.0)

            border = sorted(range(B), key=lambda bb: -nks[bb])
            for b in border:
                nk = nks[b]
                off = offs[b]
                skb = nk * P

                # ---- Phase A: QT, KT projections (head dims on partitions).
                # a-outer so each weight slice is loaded once per chunk-pair
                # (LDWEIGHTS is ~107ns per 128-col load; a-inner pays it on
                # every accumulation step).
                QT = qtp.tile([P, S], bf16, name=f"QT{b}", tag="qt")
                xq_re = xq_d[b].rearrange("(a p) s -> p a s", p=P)
                KT = ktp.tile([P, skb], bf16, name=f"KT{b}", tag="kt")
                xk_re = xk_d[:, off * P:off * P + skb].rearrange(
                    "(a p) s -> p a s", p=P
                )
                V = vp.tile([P, nk, 2, HD + 1], bf16, name=f"V{b}", tag="v")
                xv_re = xv_d[:, off * P:off * P + skb].rearrange(
                    "(a p) s -> p a s", p=P
                )

                def emit_proj_pair(w_sb, x_re, dst, chunks):
                    xss = []
                    for (s0, w) in chunks:
                        xs = xtp.tile([P, 8, 512], bf16, name="xs", tag="xt")
                        nc.sync.dma_start(xs[:, :, 0:w], x_re[:, :, s0:s0 + w])
                        xss.append(xs)
                    pj = psA.tile([P, 2, 512], f32, name="pj", tag="A")
                    for a in range(8):
                        for j, (s0, w) in enumerate(chunks):
                            nc.tensor.matmul(
                                pj[:, j, 0:w],
                                lhsT=w_sb[:, a, :],
                                rhs=xss[j][:, a, 0:w],
                                start=(a == 0),
                                stop=(a == 7),
                            )
                    for j, (s0, w) in enumerate(chunks):
                        nc.vector.tensor_copy(
                            out=dst[:, s0:s0 + w], in_=pj[:, j, 0:w]
                        )

                def emit_v_group(kts):
                    # V projection (keys on partitions, + ones column); the
                    # stationary operand is the x tile, reloaded per kt
                    for kt in kts:
                        xvt = xtp.tile([P, 8, P], bf16, name="xvt", tag="xvt")
                        nc.sync.dma_start(
                            xvt[:], xv_re[:, :, kt * P:(kt + 1) * P]
                        )
                        pv = psA.tile([P, P], f32, name="pv", tag="A")
                        for a in range(8):
                            nc.tensor.matmul(
                                pv[:],
                                lhsT=xvt[:, a, :],
                                rhs=wv[:, a, :],
                                start=(a == 0),
                                stop=(a == 7),
                            )
                        nc.vector.memset(V[:, kt, :, HD:HD + 1], 1.0)
                        nc.scalar.copy(
                            out=V[:, kt, :, 0:HD],
                            in_=pv[:].rearrange("p (h c) -> p h c", c=HD),
                        )

                # ---- Phases A+B interleaved: wide N=512 projection matmuls
                # mixed with the LDW-heavy N=128 V tiles keep PE duty (and the
                # HAM clock) up.
                qchunks = [(s0, 512) for s0 in range(0, S, 512)]
                kchunks = [
                    (s0, min(512, skb - s0)) for s0 in range(0, skb, 512)
                ]
                work = [("q", qchunks[0:2]), ("q", qchunks[2:4])]
                work += [
                    ("k", kchunks[p0:p0 + 2])
                    for p0 in range(0, len(kchunks), 2)
                ]
                vkts = list(range(nk))
                nslots = len(work)
                vgroups = [vkts[i::nslots] for i in range(nslots)]
                for (kind, chunks), vg in zip(work, vgroups):
                    if kind == "q":
                        emit_proj_pair(wq, xq_re, QT, chunks)
                    else:
                        emit_proj_pair(wk, xk_re, KT, chunks)
                    emit_v_group(vg)

                # ---- Phase C: attention for both heads, 512-wide q chunks,
                # with the output projection of each finished q chunk inlined
                # (fills the ACT-bound kt-loop gaps, spreads the out-DMA).
                OT = otp.tile([P, S], bf16, name=f"OT{b}", tag="ot")
                for qh in range(4):
                    q0 = qh * 512
                    avs = psB.tile([HD + 1, 2, 512], f32, name="avs", tag="av")
                    for kt in range(nk):
                        scs = psA.tile([P, 2, 512], f32, name="scs", tag="A")
                        for h in range(2):
                            pb = h * HD
                            nc.tensor.matmul(
                                scs[:, h, :],
                                lhsT=KT[pb:pb + HD, kt * P:(kt + 1) * P],
                                rhs=QT[pb:pb + HD, q0:q0 + 512],
                                start=True,
                                stop=True,
                            )
                        ex = expp.tile([P, 2, 512], bf16, name="ex", tag="ex")
                        nc.scalar.activation(
                            ex[:],
                            scs[:],
                            AF.Exp,
                            bias=mb[:, off + kt:off + kt + 1],
                            scale=msc[:, off + kt:off + kt + 1],
                        )
                        for h in range(2):
                            nc.tensor.matmul(
                                avs[:, h, :],
                                lhsT=V[:, kt, h, :],
                                rhs=ex[:, h, :],
                                start=(kt == 0),
                                stop=(kt == nk - 1),
                            )
                    # copy the accumulator to SBUF right away: frees the PSUM
                    # bank pair for the next q chunk's accumulation ~4us
                    # earlier than waiting out the whole normalize chain
                    avb = bcsp.tile(
                        [HD + 1, 2, 512], bf16, name="avb", tag="avb"
                    )
                    nc.vector.tensor_copy(out=avb[:], in_=avs[:])
                    # 1/denom as exp(-ln(denom)): InstReciprocal is ~6.5
                    # ns/elem on DVE and the custom-DVE fast recip doesn't
                    # encode under this walrus; two table ACT passes are 5x
                    # cheaper and accurate to ~1e-3.
                    lnd = rcpp.tile([1, 2, 512], f32, name="lnd", tag="rc")
                    nc.scalar.activation(
                        lnd[:].rearrange("p a b -> p (a b)"),
                        avb[HD:HD + 1, :, :].rearrange("p a b -> p (a b)"),
                        AF.Ln,
                    )
                    rcb = rcpp.tile([1, 2, 512], bf16, name="rcb", tag="rcb")
                    nc.scalar.activation(rcb[:], lnd[:], AF.Exp, scale=-1.0)
                    bc = psA.tile([P, 2, 512], f32, name="bc", tag="A")
                    for h in range(2):
                        nc.tensor.matmul(
                            bc[:, h, :],
                            lhsT=ones1[:],
                            rhs=rcb[:, h, :],
                            start=True,
                            stop=True,
                        )
                    bcs = bcsp.tile([P, 2, 512], bf16, name="bcs", tag="bcs")
                    if qh % 2 == 0:
                        nc.scalar.copy(out=bcs[:], in_=bc[:])
                    else:
                        nc.vector.tensor_copy(out=bcs[:], in_=bc[:])
                    nc.vector.tensor_mul(
                        out=OT[0:HD, q0:q0 + 512],
                        in0=avb[0:HD, 0, :],
                        in1=bcs[0:HD, 0, :],
                    )
                    t1 = t1p.tile([HD, 512], bf16, name="t1", tag="t1")
                    nc.vector.tensor_mul(
                        out=t1[:],
                        in0=avb[0:HD, 1, :],
                        in1=bcs[0:HD, 1, :],
                    )
                    # engines cannot write across partition bases; DMA can
                    nc.sync.dma_start(OT[HD:P, q0:q0 + 512], t1[:])

                # ---- Phase D: output projection, full 128-dim contraction
                for qt in range(16):
                    wps = psA.tile([P, D], f32, name="wps", tag="A")
                    for ch2 in range(2):
                        nc.tensor.matmul(
                            wps[:, ch2 * 512:(ch2 + 1) * 512],
                            lhsT=OT[:, qt * P:(qt + 1) * P],
                            rhs=wo[:, ch2 * 512:(ch2 + 1) * 512],
                            start=True,
                            stop=True,
                        )
                    ob = outp.tile([P, D], bf16, name="ob", tag="ob")
                    if qt % 2 == 0:
                        nc.vector.tensor_copy(out=ob[:], in_=wps[:])
                    else:
                        nc.scalar.copy(out=ob[:], in_=wps[:])
                    nc.sync.dma_start(
                        out_d[(b * 16 + qt) * P:(b * 16 + qt + 1) * P, :], ob[:]
                    )

    _split_multi_waits(nc, mybir)
    return nc


def _get_program(nks: tuple):
    if nks not in _PROG_CACHE:
        _PROG_CACHE[nks] = _build_program(nks)
    return _PROG_CACHE[nks]


def kernel(**inputs) -> np.ndarray:
    import ml_dtypes
    from concourse.bass_utils import run_bass_kernel_spmd

    bf = ml_dtypes.bfloat16

    q = np.asarray(inputs["queries"], dtype=np.float32)
    k = np.asarray(inputs["keys"], dtype=np.float32)
    v = np.asarray(inputs["values"], dtype=np.float32)
    vl = np.asarray(inputs["valid_lens"]).astype(np.int64)
    Wq = np.asarray(inputs["Wq"], dtype=np.float32)
    Wk = np.asarray(inputs["Wk"], dtype=np.float32)
    Wv = np.asarray(inputs["Wv"], dtype=np.float32)
    Wo = np.asarray(inputs["Wo"], dtype=np.float32)

    nks = tuple(
        (S // P) if int(vl[b]) == 0
        else min(S // P, int(math.ceil(int(vl[b]) / P)))
        for b in range(B)
    )
    nc = _get_program(nks)

    KT_tot = sum(nks)
    offs = [sum(nks[:b]) for b in range(B)]

    # shared across cores (host arrays reused; staging per device is free)
    xq = np.ascontiguousarray(q.transpose(0, 2, 1)).astype(bf)
    xk = np.concatenate(
        [k[b].T[:, : nks[b] * P] for b in range(B)], axis=1
    ).astype(bf)
    xv = np.concatenate(
        [v[b].T[:, : nks[b] * P] for b in range(B)], axis=1
    ).astype(bf)

    m_bias = np.empty((P, KT_tot), np.float32)
    m_scale = np.empty((P, KT_tot), np.float32)
    for b in range(B):
        vlb = int(vl[b])
        kk = (
            np.arange(nks[b])[None, :] * P + np.arange(P)[:, None]
        ).astype(np.int64)
        if vlb == 0:
            m_bias[:, offs[b]:offs[b] + nks[b]] = 0.0
            m_scale[:, offs[b]:offs[b] + nks[b]] = 0.0
        else:
            m_bias[:, offs[b]:offs[b] + nks[b]] = np.where(kk < vlb, 0.0, NEG)
            m_scale[:, offs[b]:offs[b] + nks[b]] = 1.0 / math.sqrt(HD)

    in_maps = []
    for c in range(NCORES):
        cols = slice(c * P, (c + 1) * P)
        in_maps.append(
            {
                "xq": xq,
                "xk": xk,
                "xv": xv,
                "wq": np.ascontiguousarray(Wq[:, cols]).astype(bf),
                "wk": np.ascontiguousarray(Wk[:, cols]).astype(bf),
                "wv": np.ascontiguousarray(Wv[:, cols]).astype(bf),
                "wo": np.ascontiguousarray(Wo[cols, :]).astype(bf),
                "mb": m_bias,
                "ms": m_scale,
            }
        )

    globals()["_LAST_IN_MAPS"] = in_maps
    res = run_bass_kernel_spmd(nc, in_maps, list(range(NCORES))).results

    acc = np.zeros((B * S, D), dtype=np.float32)
    for c in range(NCORES):
        acc += res[c]["out"].astype(np.float32)
    return acc.reshape(B, S, D)


# revision 24
# speedup vs baseline: 1.3818x; 1.1730x over previous
"""Multi-head attention (B=4, S=2048, D=1024, H=16) on 8 TRN2 NeuronCores.

Sharding: pure tensor-parallel over heads. Core c owns heads (2c, 2c+1) of
EVERY batch — i.e. columns [128c, 128(c+1)) of Wq/Wk/Wv and the matching
128 rows of Wo. Attention work per (batch, head) scales with
nk_b = ceil(valid_len_b / 128) key tiles, so giving each core 2 heads of
every batch balances the per-core load exactly (each core does
2 * sum_b nk_b head-keytile units) no matter how skewed valid_lens are.
Row-parallel Wo produces per-core partial outputs [B*S, D] (bf16); the
host sums the 8 partials.

Per-core dataflow, per batch b (kt counts specialized at build time):
  QT_b[d',q] = (Xq_b Wq_c)^T   bf16, head dims on partitions (h0: 0:64,
                               h1: 64:128)
  KT_b[d',k] = (Xk_b Wk_c)^T   only the first nk_b*128 key positions
  V_b[k, h, 65]                (Xv_b Wv_c) + per-head ones column (softmax
                               denominator rides along row 64 of av)
  per 512-wide q chunk:
    per kt: scoresT[k,q] for both heads (PE row groups 0/64 -> concurrent)
            ex = exp(scores * scale + maskbias)   one ACT instr, both heads
            av_h[65, q] += V_h^T-ish @ ex_h       (PSUM accumulate over kt)
    rc = 1/av[64]  (denominators, DVE reciprocal straight from PSUM)
    bc = ones^T @ rc             broadcast 1/denom to 128 partitions (PE)
    OT[0:64] = av_h0 * bc_h0 ; t1 = av_h1 * bc_h1  (DVE mul, PSUM inputs)
    OT[64:128] <- t1  via SBUF->SBUF DMA (partition move: engines cannot
                     shift partition base, DMA can)
  out_b = OT^T-slices @ Wo_c   full 128-dim contraction (both heads in one
                               accumulation-free matmul pair per q tile)

Masking is pure data: mb/ms [128, sum nk_b] hold per-key exp bias/scale;
valid_len==0 gives scale=0,bias=0 -> uniform attention over all S keys,
matching jax.nn.softmax of an all-masked row.
"""

import math

import numpy as np

B, S, D, H = 4, 2048, 1024, 16
HD = D // H  # 64
NCORES = 8
NEG = -1.0e6
P = 128

_PROG_CACHE = {}


def _patch_tile_drain():
    """The walrus build in this container rejects sem waits attached to the
    Tile end-of-kernel Drain ("Too many sync wait commands" / SIGABRT).
    Replace them with standalone EventSemaphore waits, which it accepts."""
    import concourse.tile as tile
    from concourse.vector_clock import ScopedClock

    if getattr(tile.TileContext, "_drain_patched", False):
        return

    def _drain_and_barrier(self, tick_clock, wait_clock):
        nc = self.nc
        drain_inst = nc.sync.drain()
        wait_clock.add_sem_waits(
            drain_inst.ins, ScopedClock({None: tick_clock.global_clock})
        )
        si = drain_inst.ins.sync_info
        waits = list(si.on_wait) if si is not None and si.on_wait else []
        if waits:
            si.on_wait.clear()
            by_id, by_name = {}, {}
            for h in wait_clock.sems.allocated().values():
                by_id[getattr(h, "id", None)] = h
                by_name[getattr(h, "name", None)] = h
            for w in waits:
                h = by_id.get(w.id) or by_name.get(w.ant_name)
                assert h is not None, f"no handle for sem {w.ant_name} ({w.id})"
                nc.sync.wait_ge(h, w.wait_value)
        nc.all_engine_barrier()
        assert self.sems is not None
        popped = nc._tile_sem_poison_stack.pop()
        assert popped is self._sem_poison
        nc.clear_and_free_semaphores(list(self.sems.allocated().values()))
        nc.all_engine_barrier()

    tile.TileContext._drain_and_barrier = _drain_and_barrier
    tile.TileContext._drain_patched = True


def _split_multi_waits(nc, mybir):
    """This container's walrus rejects instructions carrying more than one
    semaphore wait ("Too many sync wait commands"). Hoist excess waits into
    standalone EventSemaphore instructions on the same engine, inserted
    immediately before the instruction — same-engine stream order preserves
    the semantics exactly."""
    n_ev = 0
    for fn in nc.m.functions:
        for bb in fn.blocks:
            insts = bb.instructions
            out = []
            for inst in insts:
                si = inst.sync_info
                waits = list(si.on_wait) if si is not None and si.on_wait else []
                keep = 0 if inst.opcode == "Drain" else 1
                if len(waits) > keep:
                    excess = waits[: len(waits) - keep]
                    kept = waits[len(waits) - keep:]
                    si.on_wait.clear()
                    si.on_wait.extend(kept)
                    for w in excess:
                        ev = mybir.InstEventSemaphore(
                            name=f"{inst.name}-hw{n_ev}",
                            engine=inst.engine,
                        )
                        ev.sync_info = mybir.SyncInfo(on_wait=[w], on_update=[])
                        out.append(ev)
                        n_ev += 1
                out.append(inst)
            if n_ev:
                insts[:] = out
    return n_ev


def _build_program(nks: tuple):
    import concourse.bass as bass
    import concourse.mybir as mybir
    import concourse.tile as tile

    _patch_tile_drain()

    f32 = mybir.dt.float32
    f32r = mybir.dt.float32r
    bf16 = mybir.dt.bfloat16
    AF = mybir.ActivationFunctionType

    KT_tot = sum(nks)
    SK = KT_tot * P
    offs = [sum(nks[:b]) for b in range(B)]

    nc = bass.Bass()

    xq_d = nc.dram_tensor("xq", [B, D, S], bf16, kind="ExternalInput")
    xk_d = nc.dram_tensor("xk", [D, SK], bf16, kind="ExternalInput")
    xv_d = nc.dram_tensor("xv", [D, SK], bf16, kind="ExternalInput")
    wq_d = nc.dram_tensor("wq", [D, P], bf16, kind="ExternalInput")
    wk_d = nc.dram_tensor("wk", [D, P], bf16, kind="ExternalInput")
    wv_d = nc.dram_tensor("wv", [D, P], bf16, kind="ExternalInput")
    wo_d = nc.dram_tensor("wo", [P, D], bf16, kind="ExternalInput")
    mb_d = nc.dram_tensor("mb", [P, KT_tot], f32, kind="ExternalInput")
    ms_d = nc.dram_tensor("ms", [P, KT_tot], f32, kind="ExternalInput")
    out_d = nc.dram_tensor("out", [B * S, D], bf16, kind="ExternalOutput")

    with tile.TileContext(nc) as tc:
        with (
            tc.tile_pool(name="pp", bufs=1) as pp,
            tc.tile_pool(name="qtp", bufs=2) as qtp,
            tc.tile_pool(name="ktp", bufs=2) as ktp,
            tc.tile_pool(name="vp", bufs=2) as vp,
            tc.tile_pool(name="otp", bufs=2) as otp,
            tc.tile_pool(name="t1p", bufs=3) as t1p,
            tc.tile_pool(name="xtp", bufs=5) as xtp,
            tc.tile_pool(name="expp", bufs=3) as expp,
            tc.tile_pool(name="rcpp", bufs=2) as rcpp,
            tc.tile_pool(name="bcsp", bufs=2) as bcsp,
            tc.tile_pool(name="outp", bufs=3) as outp,
            tc.tile_pool(name="psA", bufs=3, space="PSUM") as psA,
            tc.tile_pool(name="psB", bufs=1, space="PSUM") as psB,
        ):
            # persistent: weights, masks, ones row
            wq = pp.tile([P, 8, P], bf16, name="wq")
            wk = pp.tile([P, 8, P], bf16, name="wk")
            wv = pp.tile([P, 8, P], bf16, name="wv")
            wo = pp.tile([P, D], bf16, name="wo")
            mb = pp.tile([P, KT_tot], f32, name="mb")
            msc = pp.tile([P, KT_tot], f32, name="msc")

            nc.sync.dma_start(wq[:], wq_d[:, :].rearrange("(a p) c -> p a c", p=P))
            nc.sync.dma_start(wk[:], wk_d[:, :].rearrange("(a p) c -> p a c", p=P))
            nc.sync.dma_start(wv[:], wv_d[:, :].rearrange("(a p) c -> p a c", p=P))
            nc.sync.dma_start(wo[:], wo_d[:, :])
            nc.sync.dma_start(mb[:], mb_d[:, :])
            nc.sync.dma_start(msc[:], ms_d[:, :])
            ones1 = pp.tile([1, P], bf16, name="ones1")
            nc.any.memset(ones1[:], 1.0)

            border = sorted(range(B), key=lambda bb: -nks[bb])
            for b in border:
                nk = nks[b]
                off = offs[b]
                skb = nk * P

                # ---- Phase A: QT, KT projections (head dims on partitions).
                # a-outer so each weight slice is loaded once per chunk-pair
                # (LDWEIGHTS is ~107ns per 128-col load; a-inner pays it on
                # every accumulation step).
                QT = qtp.tile([P, S], bf16, name=f"QT{b}", tag="qt")
                xq_re = xq_d[b].rearrange("(a p) s -> p a s", p=P)
                KT = ktp.tile([P, skb], bf16, name=f"KT{b}", tag="kt")
                xk_re = xk_d[:, off * P:off * P + skb].rearrange(
                    "(a p) s -> p a s", p=P
                )
                V = vp.tile([P, nk, 2, HD + 1], bf16, name=f"V{b}", tag="v")
                xv_re = xv_d[:, off * P:off * P + skb].rearrange(
                    "(a p) s -> p a s", p=P
                )

                def emit_proj_pair(w_sb, x_re, dst, chunks):
                    xss = []
                    for (s0, w) in chunks:
                        xs = xtp.tile([P, 8, 512], bf16, name="xs", tag="xt")
                        nc.sync.dma_start(xs[:, :, 0:w], x_re[:, :, s0:s0 + w])
                        xss.append(xs)
                    pj = psA.tile([P, 2, 512], f32, name="pj", tag="A")
                    for a in range(8):
                        for j, (s0, w) in enumerate(chunks):
                            nc.tensor.matmul(
                                pj[:, j, 0:w],
                                lhsT=w_sb[:, a, :],
                                rhs=xss[j][:, a, 0:w],
                                start=(a == 0),
                                stop=(a == 7),
                            )
                    for j, (s0, w) in enumerate(chunks):
                        nc.vector.tensor_copy(
                            out=dst[:, s0:s0 + w], in_=pj[:, j, 0:w]
                        )

                def emit_v_group(kts):
                    # V projection (keys on partitions, + ones column); the
                    # stationary operand is the x tile, reloaded per kt
                    for kt in kts:
                        xvt = xtp.tile([P, 8, P], bf16, name="xvt", tag="xvt")
                        nc.sync.dma_start(
                            xvt[:], xv_re[:, :, kt * P:(kt + 1) * P]
                        )
                        pv = psA.tile([P, P], f32, name="pv", tag="A")
                        for a in range(8):
                            nc.tensor.matmul(
                                pv[:],
                                lhsT=xvt[:, a, :],
                                rhs=wv[:, a, :],
                                start=(a == 0),
                                stop=(a == 7),
                            )
                        nc.vector.memset(V[:, kt, :, HD:HD + 1], 1.0)
                        nc.vector.tensor_copy(
                            out=V[:, kt, :, 0:HD],
                            in_=pv[:].rearrange("p (h c) -> p h c", c=HD),
                        )

                # ---- Phases A+B interleaved: wide N=512 projection matmuls
                # mixed with the LDW-heavy N=128 V tiles keep PE duty (and the
                # HAM clock) up.
                qchunks = [(s0, 512) for s0 in range(0, S, 512)]
                kchunks = [
                    (s0, min(512, skb - s0)) for s0 in range(0, skb, 512)
                ]
                work = [("q", qchunks[0:2]), ("q", qchunks[2:4])]
                work += [
                    ("k", kchunks[p0:p0 + 2])
                    for p0 in range(0, len(kchunks), 2)
                ]
                vkts = list(range(nk))
                nslots = len(work)
                vgroups = [vkts[i::nslots] for i in range(nslots)]
                for (kind, chunks), vg in zip(work, vgroups):
                    if kind == "q":
                        emit_proj_pair(wq, xq_re, QT, chunks)
                    else:
                        emit_proj_pair(wk, xk_re, KT, chunks)
                    emit_v_group(vg)

                # ---- Phase C: attention for both heads, 512-wide q chunks,
                # with the output projection of each finished q chunk inlined
                # (fills the ACT-bound kt-loop gaps, spreads the out-DMA).
                OT = otp.tile([P, S], bf16, name=f"OT{b}", tag="ot")
                for qh in range(4):
                    q0 = qh * 512
                    avs = psB.tile([HD + 1, 2, 512], f32, name="avs", tag="av")
                    for kt in range(nk):
                        scs = psA.tile([P, 2, 512], f32, name="scs", tag="A")
                        for h in range(2):
                            pb = h * HD
                            nc.tensor.matmul(
                                scs[:, h, :],
                                lhsT=KT[pb:pb + HD, kt * P:(kt + 1) * P],
                                rhs=QT[pb:pb + HD, q0:q0 + 512],
                                start=True,
                                stop=True,
                            )
                        ex = expp.tile([P, 2, 512], bf16, name="ex", tag="ex")
                        nc.scalar.activation(
                            ex[:],
                            scs[:],
                            AF.Exp,
                            bias=mb[:, off + kt:off + kt + 1],
                            scale=msc[:, off + kt:off + kt + 1],
                        )
                        for h in range(2):
                            nc.tensor.matmul(
                                avs[:, h, :],
                                lhsT=V[:, kt, h, :],
                                rhs=ex[:, h, :],
                                start=(kt == 0),
                                stop=(kt == nk - 1),
                            )
                    # 1/denom as exp(-ln(denom)): InstReciprocal is ~6.5
                    # ns/elem on DVE and the custom-DVE fast recip doesn't
                    # encode under this walrus; two table ACT passes are 5x
                    # cheaper and accurate to ~1e-3. Ln reads the denominator
                    # row straight from PSUM so the ACT FIFO never blocks on
                    # DVE (head-of-line stalls starve the next exp).
                    lnd = rcpp.tile([1, 2, 512], f32, name="lnd", tag="rc")
                    nc.scalar.activation(
                        lnd[:].rearrange("p a b -> p (a b)"),
                        avs[HD:HD + 1, :, :].rearrange("p a b -> p (a b)"),
                        AF.Ln,
                    )
                    # copy the accumulator to SBUF right away: frees the PSUM
                    # bank pair for the next q chunk's accumulation
                    avb = bcsp.tile(
                        [HD + 1, 2, 512], bf16, name="avb", tag="avb"
                    )
                    nc.vector.tensor_copy(out=avb[:], in_=avs[:])
                    rcb = rcpp.tile([1, 2, 512], bf16, name="rcb", tag="rcb")
                    nc.scalar.activation(rcb[:], lnd[:], AF.Exp, scale=-1.0)
                    bc = psA.tile([P, 2, 512], f32, name="bc", tag="A")
                    for h in range(2):
                        nc.tensor.matmul(
                            bc[:, h, :],
                            lhsT=ones1[:],
                            rhs=rcb[:, h, :],
                            start=True,
                            stop=True,
                        )
                    bcs = bcsp.tile([P, 2, 512], bf16, name="bcs", tag="bcs")
                    if qh % 2 == 0:
                        nc.scalar.copy(out=bcs[:], in_=bc[:])
                    else:
                        nc.vector.tensor_copy(out=bcs[:], in_=bc[:])
                    nc.vector.tensor_mul(
                        out=OT[0:HD, q0:q0 + 512],
                        in0=avb[0:HD, 0, :],
                        in1=bcs[0:HD, 0, :],
                    )
                    t1 = t1p.tile([HD, 512], bf16, name="t1", tag="t1")
                    nc.vector.tensor_mul(
                        out=t1[:],
                        in0=avb[0:HD, 1, :],
                        in1=bcs[0:HD, 1, :],
                    )
                    # engines cannot write across partition bases; DMA can
                    nc.sync.dma_start(OT[HD:P, q0:q0 + 512], t1[:])

                # ---- Phase D: output projection, full 128-dim contraction
                for qt in range(16):
                    wps = psA.tile([P, D], f32, name="wps", tag="A")
                    for ch2 in range(2):
                        nc.tensor.matmul(
                            wps[:, ch2 * 512:(ch2 + 1) * 512],
                            lhsT=OT[:, qt * P:(qt + 1) * P],
                            rhs=wo[:, ch2 * 512:(ch2 + 1) * 512],
                            start=True,
                            stop=True,
                        )
                    ob = outp.tile([P, D], bf16, name="ob", tag="ob")
                    if qt % 2 == 0:
                        nc.vector.tensor_copy(out=ob[:], in_=wps[:])
                    else:
                        nc.scalar.copy(out=ob[:], in_=wps[:])
                    nc.sync.dma_start(
                        out_d[(b * 16 + qt) * P:(b * 16 + qt + 1) * P, :], ob[:]
                    )

    _split_multi_waits(nc, mybir)
    return nc


def _get_program(nks: tuple):
    if nks not in _PROG_CACHE:
        _PROG_CACHE[nks] = _build_program(nks)
    return _PROG_CACHE[nks]


def kernel(**inputs) -> np.ndarray:
    import ml_dtypes
    from concourse.bass_utils import run_bass_kernel_spmd

    bf = ml_dtypes.bfloat16

    q = np.asarray(inputs["queries"], dtype=np.float32)
    k = np.asarray(inputs["keys"], dtype=np.float32)
    v = np.asarray(inputs["values"], dtype=np.float32)
    vl = np.asarray(inputs["valid_lens"]).astype(np.int64)
    Wq = np.asarray(inputs["Wq"], dtype=np.float32)
    Wk = np.asarray(inputs["Wk"], dtype=np.float32)
    Wv = np.asarray(inputs["Wv"], dtype=np.float32)
    Wo = np.asarray(inputs["Wo"], dtype=np.float32)

    nks = tuple(
        (S // P) if int(vl[b]) == 0
        else min(S // P, int(math.ceil(int(vl[b]) / P)))
        for b in range(B)
    )
    nc = _get_program(nks)

    KT_tot = sum(nks)
    offs = [sum(nks[:b]) for b in range(B)]

    # shared across cores (host arrays reused; staging per device is free)
    xq = np.ascontiguousarray(q.transpose(0, 2, 1)).astype(bf)
    xk = np.concatenate(
        [k[b].T[:, : nks[b] * P] for b in range(B)], axis=1
    ).astype(bf)
    xv = np.concatenate(
        [v[b].T[:, : nks[b] * P] for b in range(B)], axis=1
    ).astype(bf)

    m_bias = np.empty((P, KT_tot), np.float32)
    m_scale = np.empty((P, KT_tot), np.float32)
    for b in range(B):
        vlb = int(vl[b])
        kk = (
            np.arange(nks[b])[None, :] * P + np.arange(P)[:, None]
        ).astype(np.int64)
        if vlb == 0:
            m_bias[:, offs[b]:offs[b] + nks[b]] = 0.0
            m_scale[:, offs[b]:offs[b] + nks[b]] = 0.0
        else:
            m_bias[:, offs[b]:offs[b] + nks[b]] = np.where(kk < vlb, 0.0, NEG)
            m_scale[:, offs[b]:offs[b] + nks[b]] = 1.0 / math.sqrt(HD)

    in_maps = []
    for c in range(NCORES):
        cols = slice(c * P, (c + 1) * P)
        in_maps.append(
            {
                "xq": xq,
                "xk": xk,
                "xv": xv,
                "wq": np.ascontiguousarray(Wq[:, cols]).astype(bf),
                "wk": np.ascontiguousarray(Wk[:, cols]).astype(bf),
                "wv": np.ascontiguousarray(Wv[:, cols]).astype(bf),
                "wo": np.ascontiguousarray(Wo[cols, :]).astype(bf),
                "mb": m_bias,
                "ms": m_scale,
            }
        )

    globals()["_LAST_IN_MAPS"] = in_maps
    res = run_bass_kernel_spmd(nc, in_maps, list(range(NCORES))).results

    acc = np.zeros((B * S, D), dtype=np.float32)
    for c in range(NCORES):
        acc += res[c]["out"].astype(np.float32)
    return acc.reshape(B, S, D)
